# revision 18
# baseline (speedup 1.0000x reference)
"""MultiHeadAttention Trainium2 Bass kernel.

Head-sharded tensor parallel across 8 NeuronCores (2 heads/core).
All-transposed dataflow: activations live feature-on-partition so no
on-device activation transposes are needed; the per-head attention
computes S.T = K Q.T directly, softmax is max-free (scores are bounded),
the additive attention bias is applied as a multiply by exp(bias)
(precomputed on host, with key-padding-masked rows zeroed per batch so
no separate mask pass is needed), and the denominator falls out of the
PV matmul via an all-ones 65th lhsT column.

Host side: inputs are pre-transposed / pre-cast to fp16, outputs are
partial sums (row-parallel out projection) summed on host.
"""

import sys

sys.path.insert(0, "/opt/trn_rl_repo")

import numpy as np

B, S, H, NH = 2, 2048, 1024, 16
HD = H // NH            # 64
NCORES = 8
HPC = NH // NCORES      # 2 heads per core
CW = HPC * HD           # 128 = per-core slice width
R = B * S               # 4096 flattened rows
SCALE = float(HD) ** -0.5
F = H // 128            # 8 feature blocks
RC = R // 512           # 8 row chunks
QC = S // 512           # 4 q chunks per batch
KB = S // 128           # 16 k blocks per batch
T = B * KB              # 32 (b, kb) blocks

_CACHE = {}


def _build_module():
    import concourse.bass as bass
    import concourse.tile as tile
    from concourse import bacc, mybir
    from concourse.masks import make_identity

    f16 = mybir.dt.float16
    f32 = mybir.dt.float32
    Exp = mybir.ActivationFunctionType.Exp

    nc = bacc.Bacc(
        "TRN2", target_bir_lowering=False, debug=False, num_devices=NCORES
    )

    # ---- DRAM I/O (per core) ----
    xq = nc.dram_tensor("xq_t", [H, R], f16, kind="ExternalInput").ap()
    xk = nc.dram_tensor("xk_t", [H, R], f16, kind="ExternalInput").ap()
    xv = nc.dram_tensor("xv_t", [H, R], f16, kind="ExternalInput").ap()
    wq = nc.dram_tensor("wq_t", [H, CW], f16, kind="ExternalInput").ap()
    wk = nc.dram_tensor("wk_t", [H, CW], f16, kind="ExternalInput").ap()
    wv = nc.dram_tensor("wv_t", [H, CW], f16, kind="ExternalInput").ap()
    wo = nc.dram_tensor("wo_t", [CW, H], f16, kind="ExternalInput").ap()
    qb = nc.dram_tensor("qb_col", [CW, 1], f32, kind="ExternalInput").ap()
    kb_ = nc.dram_tensor("kb_col", [CW, 1], f32, kind="ExternalInput").ap()
    eb = nc.dram_tensor("eb_t", [QC, S, HPC * 512], f16,
                        kind="ExternalInput").ap()
    m01f = nc.dram_tensor("m01_f32", [128, T], f32, kind="ExternalInput").ap()
    m01h = nc.dram_tensor("m01_v", [128, T], f16, kind="ExternalInput").ap()
    opart = nc.dram_tensor("o_part", [R, H], f16, kind="ExternalOutput").ap()

    with tile.TileContext(nc) as tc:
        _emit(tc, nc, f16, f32, Exp, make_identity, bass,
              xq, xk, xv, wq, wk, wv, wo, qb, kb_, eb, m01f, m01h, opart)

    nc.compile()
    return nc


def _emit(tc, nc, f16, f32, Exp, make_identity, bass,
          xq, xk, xv, wq, wk, wv, wo, qb, kb_, eb, m01f, m01h, opart):
    from contextlib import ExitStack

    with ExitStack() as top:
        consts = top.enter_context(tc.tile_pool(name="consts", bufs=1))
        pers = top.enter_context(tc.tile_pool(name="pers", bufs=1))
        xpool = top.enter_context(tc.tile_pool(name="xin", bufs=3))
        mm = top.enter_context(tc.tile_pool(name="mmpsum", bufs=2,
                                            space="PSUM"))
        pp = top.enter_context(tc.tile_pool(name="projpsum", bufs=2,
                                            space="PSUM"))
        cvp_pool = top.enter_context(tc.tile_pool(name="cvpsum", bufs=1,
                                                  space="PSUM"))
        vtp = top.enter_context(tc.tile_pool(name="vt", bufs=2))
        ebp = top.enter_context(tc.tile_pool(name="ebp", bufs=3))
        esp = top.enter_context(tc.tile_pool(name="esp", bufs=3))
        ptp = top.enter_context(tc.tile_pool(name="ptp", bufs=3))
        bcp = top.enter_context(tc.tile_pool(name="bcp", bufs=2))
        rcp = top.enter_context(tc.tile_pool(name="rcp", bufs=1))
        op = top.enter_context(tc.tile_pool(name="op", bufs=2))
        dscr = top.enter_context(tc.tile_pool(name="dscr", bufs=4,
                                              space="DRAM"))

        # ---- tiles for constants / persistent activations ----
        wq_sb = consts.tile([128, F, 128], f16, tag="wq")
        wk_sb = consts.tile([128, F, 128], f16, tag="wk")
        wv_sb = consts.tile([128, F, 128], f16, tag="wv")
        wo_sb = consts.tile([128, H], f16, tag="wo")
        qb_sb = consts.tile([128, 1], f32, tag="qb")
        kb_sb = consts.tile([128, 1], f32, tag="kb")
        m01f_sb = consts.tile([128, T], f32, tag="m01f")
        ident = consts.tile([128, 128], f16, tag="ident")

        # Per-head q tiles, zero-padded on the other head's partitions so
        # the scores matmul contracts over all 128 partitions (64-partition
        # matmuls stream rhs at half rate on TRN2).
        qT_h = [pers.tile([128, R], f16, tag=f"qT{h}", name=f"qT{h}")
                for h in range(HPC)]
        kT_sb = pers.tile([128, R], f16, tag="kT")
        # v in natural layout per (b,kb) block: 64 v cols (zeroed at
        # key-padding-masked rows) + 0/1 mask col (row 64 of the PV
        # output = masked softmax denominator).
        v_nat = pers.tile([128, T, HPC, 65], f16, tag="vn")
        ctxn = [pers.tile([128, S], f16, tag=f"ctxn{b}", name=f"ctxn{b}")
                for b in range(B)]
        ctx1 = [pers.tile([64, S], f16, tag=f"ctx1{b}", name=f"ctx1{b}")
                for b in range(B)]

        opr = opart.rearrange("(g p) hh -> p g hh", p=128)
        ebr = eb.rearrange("qc (kb p) m -> p qc kb m", p=128)
        xqr = xq.rearrange("(f p) r -> p f r", p=128)
        xkr = xk.rearrange("(f p) r -> p f r", p=128)
        xvr = xv.rearrange("(f p) r -> p f r", p=128)
        PIPE = 1            # pending 2-kb groups before PV drain

        nc.vector.memset(qT_h[0][64:128, :], 0.0)
        nc.vector.memset(qT_h[1][0:64, :], 0.0)
        make_identity(nc, ident)
        nc.sync.dma_start(m01f_sb, m01f)
        for h in range(HPC):
            nc.sync.dma_start(v_nat[:, :, h, 64:65], m01h)

        # ---------- filler pump: PE work interleaved into attention ----
        # Each unit is {"dma": fn|None, "work": fn}. DMAs run DMA_LEAD
        # units ahead of work so filler matmuls never stall the PE queue.
        units = []
        state = {"d": 0, "w": 0}
        DMA_LEAD = 2

        def _pump_dmas():
            while state["d"] < min(len(units), state["w"] + DMA_LEAD + 1):
                u = units[state["d"]]
                if u["dma"] is not None:
                    u["dma"]()
                state["d"] += 1

        def pump(n=1):
            _pump_dmas()
            for _ in range(n):
                if state["w"] >= len(units):
                    return
                units[state["w"]]["work"]()
                state["w"] += 1
                _pump_dmas()

        def drain():
            while state["w"] < len(units):
                pump(1)

        # ---------- projection units (one rc chunk each) ----------
        def proj_unit(which, rc):
            w_sb, xr, bias_col = {
                "q": (wq_sb, xqr, qb_sb),
                "k": (wk_sb, xkr, kb_sb),
                "v": (wv_sb, xvr, None),
            }[which]
            box = {}

            def dma():
                xt = xpool.tile([128, F, 512], f16, tag="xt",
                                name=f"xt_{which}{rc}")
                nc.sync.dma_start(xt, xr[:, :, rc * 512:(rc + 1) * 512])
                box["xt"] = xt

            def work():
                xt = box["xt"]
                ps = pp.tile([128, 512], f32, tag="pp",
                             name=f"ps_{which}{rc}")
                for f in range(F):
                    nc.tensor.matmul(ps, lhsT=w_sb[:, f, :], rhs=xt[:, f, :],
                                     start=(f == 0), stop=(f == F - 1))
                cols = slice(rc * 512, (rc + 1) * 512)
                if which == "k":
                    nc.vector.tensor_scalar_add(kT_sb[:, cols], ps, bias_col)
                elif which == "q":
                    for h in range(HPC):
                        rows = slice(h * 64, (h + 1) * 64)
                        nc.vector.tensor_scalar_add(
                            qT_h[h][rows, cols], ps[rows, :],
                            bias_col[rows, :])
                else:
                    vt = vtp.tile([128, 512], f16, tag="vt")
                    nc.vector.tensor_copy(vt, ps)
                    for i in range(4):
                        t = rc * 4 + i          # t = b*KB + kb
                        tp = pp.tile([128, 128], f16, tag="pp",
                                     name=f"tp{rc}_{i}")
                        nc.tensor.transpose(
                            tp, vt[:, i * 128:(i + 1) * 128], ident)
                        for h in range(HPC):
                            nc.vector.tensor_scalar_mul(
                                v_nat[:, t, h, 0:64],
                                tp[:, h * 64:(h + 1) * 64],
                                m01f_sb[:, t:t + 1])
            return {"dma": dma, "work": work}

        # ---------- out-projection units (per (qc, b): 4 ri + store) ----
        def op_units(qc, b):
            box = {}

            def ri_work(ri, qc=qc, b=b):
                if ri == 0:
                    box["ob"] = op.tile([128, QC, H], f16, tag="ob",
                                        name=f"ob{qc}_{b}")
                ob_g = box["ob"]
                rb = qc * QC + ri
                lhsT = ctxn[b][:, rb * 128:(rb + 1) * 128]
                for h in range(HPC):
                    po = pp.tile([128, 512], f32, tag="pp",
                                 name=f"po{qc}_{b}_{ri}_{h}")
                    nc.tensor.matmul(po, lhsT=lhsT,
                                     rhs=wo_sb[:, h * 512:(h + 1) * 512],
                                     start=True, stop=True)
                    nc.vector.tensor_copy(
                        ob_g[:, ri, h * 512:(h + 1) * 512], po)

            def store(qc=qc, b=b):
                g0 = b * (S // 128) + qc * QC
                nc.sync.dma_start(opr[:, g0:g0 + QC, :], box["ob"])

            out = [{"dma": None, "work": (lambda ri=ri: ri_work(ri))}
                   for ri in range(QC)]
            out.append({"dma": None, "work": store})
            return out

        # ---------- eb prefetch (half-chunks of 8 kb, shared by b) ----
        def ebq_half(qc, half):
            t = ebp.tile([128, KB // 2, HPC, 512], f16, tag="eb",
                         name=f"ebq{qc}_{half}")
            for g in range(2):
                k0 = half * 8 + g * 4
                nc.sync.dma_start(t[:, g * 4:(g + 1) * 4, :, :],
                                  ebr[:, qc, k0:k0 + 4, :])
            return t

        # ---------- attention chunk ----------
        def attn(qc, b, eb_halves, next_half):
            cvp = cvp_pool.tile([65, HPC, 512], f32, tag="cv",
                                name=f"cv{qc}_{b}")

            def emit_pv(ptt, g):
                for j in range(2):
                    kb = 2 * g + j
                    for h in range(HPC):
                        nc.tensor.matmul(
                            cvp[:, h, :],
                            lhsT=v_nat[:, b * KB + kb, h, :],
                            rhs=ptt[:, j, h, :],
                            start=(kb == 0), stop=(kb == KB - 1))

            pend = []
            est = None
            for kb in range(KB):
                sps = mm.tile([128, HPC, 512], f32, tag="sps",
                              name=f"sps{qc}_{kb}_{b}")
                for h in range(HPC):
                    nc.tensor.matmul(
                        sps[:, h, :],
                        lhsT=kT_sb[:, b * S + kb * 128:b * S + (kb + 1) * 128],
                        rhs=qT_h[h][:, b * S + qc * 512:b * S + (qc + 1) * 512],
                        start=True, stop=True)
                g, half = kb // 2, kb % 2
                if half == 0:
                    est = esp.tile([128, 2, HPC, 512], f16, tag="es",
                                   name=f"es{qc}_{b}_{g}")
                nc.scalar.activation(est[:, half], sps, func=Exp, scale=SCALE)
                if half == 1:
                    ptt = ptp.tile([128, 2, HPC, 512], f16, tag="pt",
                                   name=f"pt{qc}_{b}_{g}")
                    ebt = eb_halves[g // 4]
                    gg = g % 4
                    eng = nc.gpsimd if g % 2 == 1 else nc.vector
                    eng.tensor_mul(
                        ptt.rearrange("p a h m -> p (a h m)"), est.rearrange(
                            "p a h m -> p (a h m)"),
                        ebt[:, 2 * gg:2 * gg + 2, :, :].rearrange(
                            "p a h m -> p (a h m)"))
                    pend.append((ptt, g))
                    if len(pend) > PIPE:
                        emit_pv(*pend.pop(0))
                if kb == 8 and next_half is not None:
                    next_half()
                pump(1)
            for args in pend:
                emit_pv(*args)

            # evacuate ctx rows to SBUF immediately: frees the cvp bank
            # pair well before the broadcast round-trip completes, so the
            # next chunk's PV accumulation is not serialized behind it.
            cvs = bcp.tile([64, HPC, 512], f32, tag="cvs",
                           name=f"cvs{qc}_{b}")
            nc.vector.tensor_copy(cvs, cvp[0:64, :, :])
            # 1/denominator on DVE (keeps the scalar engine pure-Exp:
            # any other activation function would thrash the act table)
            rc_sb = rcp.tile([65, HPC, 512], f32, tag="rc")
            nc.vector.reciprocal(rc_sb[64:65, :, :], cvp[64:65, :, :])
            scr = dscr.tile([1, HPC, 512], f32, tag="scr",
                            name=f"scr{qc}_{b}")
            nc.sync.dma_start(scr, rc_sb[64:65, :, :])
            bc = bcp.tile([64, HPC, 512], f32, tag="bc")
            nc.sync.dma_start(bc, scr.to_broadcast((64, HPC, 512)))
            nc.vector.tensor_mul(
                ctxn[b][0:64, qc * 512:(qc + 1) * 512], cvs[:, 0, :],
                bc[:, 0, :])
            # h1: lanes 0-63; via ctx1, relocated to partitions 64-127
            nc.vector.tensor_mul(
                ctx1[b][:, qc * 512:(qc + 1) * 512], cvs[:, 1, :],
                bc[:, 1, :])
            nc.sync.dma_start(
                ctxn[b][64:128, qc * 512:(qc + 1) * 512],
                ctx1[b][:, qc * 512:(qc + 1) * 512])

        # ---------- schedule ----------
        nc.sync.dma_start(wq_sb, wq.rearrange("(f p) j -> p f j", p=128))
        nc.sync.dma_start(qb_sb, qb)
        nc.sync.dma_start(wk_sb, wk.rearrange("(f p) j -> p f j", p=128))
        nc.sync.dma_start(kb_sb, kb_)
        nc.sync.dma_start(wv_sb, wv.rearrange("(f p) j -> p f j", p=128))
        nc.sync.dma_start(wo_sb, wo)

        # bootstrap: just enough projection for attn(0,0) to start
        boot = [proj_unit("q", 0), proj_unit("k", 0), proj_unit("v", 0),
                proj_unit("k", 1), proj_unit("v", 1)]
        for u in boot[:2]:
            u["dma"]()
        for i, u in enumerate(boot):
            if i + 2 < len(boot):
                boot[i + 2]["dma"]()
            u["work"]()

        # filler queues per chunk
        units.extend([proj_unit("k", 2), proj_unit("v", 2),
                      proj_unit("k", 3), proj_unit("v", 3),
                      proj_unit("k", 4), proj_unit("v", 4),
                      proj_unit("k", 5), proj_unit("v", 5),
                      proj_unit("q", 4)])
        c1_fill = [proj_unit("k", 6), proj_unit("v", 6),
                   proj_unit("k", 7), proj_unit("v", 7),
                   proj_unit("q", 1), proj_unit("q", 2), proj_unit("q", 3)]
        c2_fill = [proj_unit("q", 5), proj_unit("q", 6), proj_unit("q", 7)]

        chunks = [(qc, b) for qc in range(QC) for b in range(B)]
        halves = {}
        halves[(0, 0)] = ebq_half(0, 0)
        halves[(0, 1)] = ebq_half(0, 1)

        for ci, (qc, b) in enumerate(chunks):
            if ci == 1:
                units.extend(c1_fill)
            elif ci == 2:
                units.extend(c2_fill)
            if ci >= 2:
                units.extend(op_units(*chunks[ci - 2]))

            def next_half(qc=qc, b=b):
                # during (qc, 1) prefetch the next qc's first half
                if b == 1 and qc + 1 < QC:
                    halves[(qc + 1, 0)] = ebq_half(qc + 1, 0)
            attn(qc, b, [halves[(qc, 0)], halves[(qc, 1)]], next_half)
            if b == 1 and qc + 1 < QC:
                halves[(qc + 1, 1)] = ebq_half(qc + 1, 1)

        units.extend(op_units(*chunks[-2]))
        units.extend(op_units(*chunks[-1]))
        drain()


def get_module():
    if "nc" not in _CACHE:
        _CACHE["nc"] = _build_module()
    return _CACHE["nc"]


def make_in_maps(query, key, value, key_padding_mask, bias,
                 q_w, q_b, k_w, k_b, v_w, v_b, o_w, o_b):
    f16 = np.float16
    xq_t = np.ascontiguousarray(query.reshape(R, H).T).astype(f16)
    xk_t = np.ascontiguousarray(key.reshape(R, H).T).astype(f16)
    xv_t = np.ascontiguousarray(value.reshape(R, H).T).astype(f16)

    kpm = np.asarray(key_padding_mask)  # [B, S] bool
    # m01[p, t]: 0.0 where key-padding-masked, column t = b*KB + kb
    m01 = np.empty((128, T), np.float32)
    for b in range(B):
        for kb in range(KB):
            m01[:, b * KB + kb] = np.where(kpm[b, kb * 128:(kb + 1) * 128],
                                           0.0, 1.0)
    m01_f32 = np.ascontiguousarray(m01)
    m01_v = m01.astype(f16)

    in_maps = []
    for c in range(NCORES):
        hs = slice(c * CW, (c + 1) * CW)
        # eb layout [qc, k, i, qi]: exp(bias).T pre-sliced by q chunk
        ebt = np.empty((QC, S, HPC, 512), f16)
        for i in range(HPC):
            h = c * HPC + i
            e = np.exp(np.asarray(bias[0, h], np.float32).T)  # [k, q]
            ebt[:, :, i, :] = e.reshape(S, QC, 512).transpose(1, 0, 2)
        ebt = ebt.reshape(QC, S, HPC * 512)
        in_maps.append({
            "xq_t": xq_t, "xk_t": xk_t, "xv_t": xv_t,
            "wq_t": np.ascontiguousarray(np.asarray(q_w)[hs].T).astype(f16),
            "wk_t": np.ascontiguousarray(np.asarray(k_w)[hs].T).astype(f16),
            "wv_t": np.ascontiguousarray(np.asarray(v_w)[hs].T).astype(f16),
            "wo_t": np.ascontiguousarray(np.asarray(o_w)[:, hs].T).astype(f16),
            "qb_col": np.asarray(q_b, np.float32)[hs].reshape(CW, 1).copy(),
            "kb_col": np.asarray(k_b, np.float32)[hs].reshape(CW, 1).copy(),
            "eb_t": ebt,
            "m01_f32": m01_f32,
            "m01_v": m01_v,
        })
    return in_maps


def assemble_output(results, v_b, o_w, o_b):
    acc = np.zeros((R, H), np.float32)
    for res in results:
        acc += np.asarray(res["o_part"], np.float32)
    corr = np.asarray(v_b, np.float32) @ np.asarray(o_w, np.float32).T \
        + np.asarray(o_b, np.float32)
    acc += corr[None, :]
    return acc.reshape(B, S, H).astype(np.float32)


def kernel(**inputs):
    from concourse.bass_utils import run_bass_kernel_spmd

    nc = get_module()
    in_maps = make_in_maps(**inputs)
    res = run_bass_kernel_spmd(nc, in_maps, list(range(NCORES)))
    return assemble_output(res.results, inputs["v_b"], inputs["o_w"],
                           inputs["o_b"])


# revision 19
# speedup vs baseline: 1.2692x; 1.2692x over previous
"""MultiHeadAttention Trainium2 Bass kernel.

Head-sharded tensor parallel across 8 NeuronCores (2 heads/core).
All-transposed dataflow: activations live feature-on-partition so no
on-device activation transposes are needed; the per-head attention
computes S.T = K Q.T directly, softmax is max-free (scores are bounded),
the additive attention bias is applied as a multiply by exp(bias)
(precomputed on host, with key-padding-masked rows zeroed per batch so
no separate mask pass is needed), and the denominator falls out of the
PV matmul via an all-ones 65th lhsT column.

Host side: inputs are pre-transposed / pre-cast to fp16, outputs are
partial sums (row-parallel out projection) summed on host.
"""

import sys

sys.path.insert(0, "/opt/trn_rl_repo")

import numpy as np

B, S, H, NH = 2, 2048, 1024, 16
HD = H // NH            # 64
NCORES = 8
HPC = NH // NCORES      # 2 heads per core
CW = HPC * HD           # 128 = per-core slice width
R = B * S               # 4096 flattened rows
SCALE = float(HD) ** -0.5
F = H // 128            # 8 feature blocks
RC = R // 512           # 8 row chunks
QC = S // 512           # 4 q chunks per batch
KB = S // 128           # 16 k blocks per batch
T = B * KB              # 32 (b, kb) blocks

_CACHE = {}


def _build_module():
    import concourse.bass as bass
    import concourse.tile as tile
    from concourse import bacc, mybir
    from concourse.masks import make_identity

    f16 = mybir.dt.float16
    f32 = mybir.dt.float32
    Exp = mybir.ActivationFunctionType.Exp

    nc = bacc.Bacc(
        "TRN2", target_bir_lowering=False, debug=False, num_devices=NCORES
    )

    # ---- DRAM I/O (per core) ----
    xq = nc.dram_tensor("xq_t", [H, R], f16, kind="ExternalInput").ap()
    xk = nc.dram_tensor("xk_t", [H, R], f16, kind="ExternalInput").ap()
    xv = nc.dram_tensor("xv_t", [H, R], f16, kind="ExternalInput").ap()
    wq = nc.dram_tensor("wq_t", [H, CW], f16, kind="ExternalInput").ap()
    wk = nc.dram_tensor("wk_t", [H, CW], f16, kind="ExternalInput").ap()
    wv = nc.dram_tensor("wv_t", [H, CW], f16, kind="ExternalInput").ap()
    wo = nc.dram_tensor("wo_t", [CW, H], f16, kind="ExternalInput").ap()
    qb = nc.dram_tensor("qb_col", [CW, 1], f32, kind="ExternalInput").ap()
    kb_ = nc.dram_tensor("kb_col", [CW, 1], f32, kind="ExternalInput").ap()
    eb = nc.dram_tensor("eb_t", [QC, S, HPC * 512], f16,
                        kind="ExternalInput").ap()
    m01f = nc.dram_tensor("m01_f32", [128, T], f32, kind="ExternalInput").ap()
    m01h = nc.dram_tensor("m01_v", [128, T], f16, kind="ExternalInput").ap()
    opart = nc.dram_tensor("o_part", [R, H], f16, kind="ExternalOutput").ap()

    with tile.TileContext(nc) as tc:
        _emit(tc, nc, f16, f32, Exp, make_identity, bass,
              xq, xk, xv, wq, wk, wv, wo, qb, kb_, eb, m01f, m01h, opart)

    nc.compile()
    return nc


def _emit(tc, nc, f16, f32, Exp, make_identity, bass,
          xq, xk, xv, wq, wk, wv, wo, qb, kb_, eb, m01f, m01h, opart):
    from contextlib import ExitStack

    with ExitStack() as top:
        consts = top.enter_context(tc.tile_pool(name="consts", bufs=1))
        pers = top.enter_context(tc.tile_pool(name="pers", bufs=1))
        xpool = top.enter_context(tc.tile_pool(name="xin", bufs=3))
        mm = top.enter_context(tc.tile_pool(name="mmpsum", bufs=2,
                                            space="PSUM"))
        pp = top.enter_context(tc.tile_pool(name="projpsum", bufs=2,
                                            space="PSUM"))
        cvp_pool = top.enter_context(tc.tile_pool(name="cvpsum", bufs=1,
                                                  space="PSUM"))
        vtp = top.enter_context(tc.tile_pool(name="vt", bufs=2))
        ebp = top.enter_context(tc.tile_pool(name="ebp", bufs=3))
        esp = top.enter_context(tc.tile_pool(name="esp", bufs=3))
        ptp = top.enter_context(tc.tile_pool(name="ptp", bufs=3))
        bcp = top.enter_context(tc.tile_pool(name="bcp", bufs=2))
        rcp = top.enter_context(tc.tile_pool(name="rcp", bufs=1))
        op = top.enter_context(tc.tile_pool(name="op", bufs=2))
        dscr = top.enter_context(tc.tile_pool(name="dscr", bufs=4,
                                              space="DRAM"))

        # ---- tiles for constants / persistent activations ----
        wq_sb = consts.tile([128, F, 128], f16, tag="wq")
        wk_sb = consts.tile([128, F, 128], f16, tag="wk")
        wv_sb = consts.tile([128, F, 128], f16, tag="wv")
        wo_sb = consts.tile([128, H], f16, tag="wo")
        qb_sb = consts.tile([128, 1], f32, tag="qb")
        kb_sb = consts.tile([128, 1], f32, tag="kb")
        m01f_sb = consts.tile([128, T], f32, tag="m01f")
        ident = consts.tile([128, 128], f16, tag="ident")

        # Per-head q tiles, zero-padded on the other head's partitions so
        # the scores matmul contracts over all 128 partitions (64-partition
        # matmuls stream rhs at half rate on TRN2).
        qT_h = [pers.tile([128, R], f16, tag=f"qT{h}", name=f"qT{h}")
                for h in range(HPC)]
        kT_sb = pers.tile([128, R], f16, tag="kT")
        # v in natural layout per (b,kb) block: 64 v cols (zeroed at
        # key-padding-masked rows) + 0/1 mask col (row 64 of the PV
        # output = masked softmax denominator).
        v_nat = pers.tile([128, T, HPC, 65], f16, tag="vn")
        ctxn = [pers.tile([128, S], f16, tag=f"ctxn{b}", name=f"ctxn{b}")
                for b in range(B)]
        ctx1 = [pers.tile([64, S], f16, tag=f"ctx1{b}", name=f"ctx1{b}")
                for b in range(B)]

        opr = opart.rearrange("(g p) hh -> p g hh", p=128)
        ebr = eb.rearrange("qc (kb p) m -> p qc kb m", p=128)
        xqr = xq.rearrange("(f p) r -> p f r", p=128)
        xkr = xk.rearrange("(f p) r -> p f r", p=128)
        xvr = xv.rearrange("(f p) r -> p f r", p=128)
        PIPE = 1            # pending 2-kb groups before PV drain

        nc.vector.memset(qT_h[0][64:128, :], 0.0)
        nc.vector.memset(qT_h[1][0:64, :], 0.0)
        make_identity(nc, ident)
        nc.sync.dma_start(m01f_sb, m01f)
        for h in range(HPC):
            nc.sync.dma_start(v_nat[:, :, h, 64:65], m01h)

        # ---------- filler pump: PE work interleaved into attention ----
        # Each unit is {"dma": fn|None, "work": fn}. DMAs run DMA_LEAD
        # units ahead of work so filler matmuls never stall the PE queue.
        units = []
        state = {"d": 0, "w": 0}
        DMA_LEAD = 2

        def _pump_dmas():
            while state["d"] < min(len(units), state["w"] + DMA_LEAD + 1):
                u = units[state["d"]]
                if u["dma"] is not None:
                    u["dma"]()
                state["d"] += 1

        def pump(n=1):
            _pump_dmas()
            for _ in range(n):
                if state["w"] >= len(units):
                    return
                units[state["w"]]["work"]()
                state["w"] += 1
                _pump_dmas()

        def drain():
            while state["w"] < len(units):
                pump(1)

        # ---------- projection units (one rc chunk each) ----------
        def proj_unit(which, rc):
            w_sb, xr, bias_col = {
                "q": (wq_sb, xqr, qb_sb),
                "k": (wk_sb, xkr, kb_sb),
                "v": (wv_sb, xvr, None),
            }[which]
            box = {}

            def dma():
                xt = xpool.tile([128, F, 512], f16, tag="xt",
                                name=f"xt_{which}{rc}")
                nc.sync.dma_start(xt, xr[:, :, rc * 512:(rc + 1) * 512])
                box["xt"] = xt

            def work():
                xt = box["xt"]
                ps = pp.tile([128, 512], f32, tag="pp",
                             name=f"ps_{which}{rc}")
                for f in range(F):
                    nc.tensor.matmul(ps, lhsT=w_sb[:, f, :], rhs=xt[:, f, :],
                                     start=(f == 0), stop=(f == F - 1))
                cols = slice(rc * 512, (rc + 1) * 512)
                if which == "k":
                    nc.vector.tensor_scalar_add(kT_sb[:, cols], ps, bias_col)
                elif which == "q":
                    for h in range(HPC):
                        rows = slice(h * 64, (h + 1) * 64)
                        nc.vector.tensor_scalar_add(
                            qT_h[h][rows, cols], ps[rows, :],
                            bias_col[rows, :])
                else:
                    vt = vtp.tile([128, 512], f16, tag="vt")
                    nc.scalar.copy(vt, ps)
                    for i in range(4):
                        t = rc * 4 + i          # t = b*KB + kb
                        tp = pp.tile([128, 128], f16, tag="pp",
                                     name=f"tp{rc}_{i}")
                        nc.tensor.transpose(
                            tp, vt[:, i * 128:(i + 1) * 128], ident)
                        for h in range(HPC):
                            nc.vector.tensor_scalar_mul(
                                v_nat[:, t, h, 0:64],
                                tp[:, h * 64:(h + 1) * 64],
                                m01f_sb[:, t:t + 1])
            return {"dma": dma, "work": work}

        # ---------- out-projection units (per (qc, b): 4 ri + store) ----
        def op_units(qc, b):
            box = {}

            def ri_work(ri, qc=qc, b=b):
                if ri == 0:
                    box["ob"] = op.tile([128, QC, H], f16, tag="ob",
                                        name=f"ob{qc}_{b}")
                ob_g = box["ob"]
                rb = qc * QC + ri
                lhsT = ctxn[b][:, rb * 128:(rb + 1) * 128]
                for h in range(HPC):
                    po = pp.tile([128, 512], f32, tag="pp",
                                 name=f"po{qc}_{b}_{ri}_{h}")
                    nc.tensor.matmul(po, lhsT=lhsT,
                                     rhs=wo_sb[:, h * 512:(h + 1) * 512],
                                     start=True, stop=True)
                    evac = nc.scalar.copy if h == 0 \
                        else nc.vector.tensor_copy
                    evac(ob_g[:, ri, h * 512:(h + 1) * 512], po)

            def store(qc=qc, b=b):
                g0 = b * (S // 128) + qc * QC
                nc.sync.dma_start(opr[:, g0:g0 + QC, :], box["ob"])

            out = [{"dma": None, "work": (lambda ri=ri: ri_work(ri))}
                   for ri in range(QC)]
            out.append({"dma": None, "work": store})
            return out

        # ---------- eb prefetch (half-chunks of 8 kb, shared by b) ----
        def ebq_half(qc, half):
            t = ebp.tile([128, KB // 2, HPC, 512], f16, tag="eb",
                         name=f"ebq{qc}_{half}")
            for g in range(2):
                k0 = half * 8 + g * 4
                nc.sync.dma_start(t[:, g * 4:(g + 1) * 4, :, :],
                                  ebr[:, qc, k0:k0 + 4, :])
            return t

        # ---------- attention chunk ----------
        def attn(qc, b, eb_halves, next_half):
            cvp = cvp_pool.tile([65, HPC, 512], f32, tag="cv",
                                name=f"cv{qc}_{b}")

            def emit_pv(ptt, g):
                for j in range(2):
                    kb = 2 * g + j
                    for h in range(HPC):
                        nc.tensor.matmul(
                            cvp[:, h, :],
                            lhsT=v_nat[:, b * KB + kb, h, :],
                            rhs=ptt[:, j, h, :],
                            start=(kb == 0), stop=(kb == KB - 1))

            pend = []
            est = None
            for kb in range(KB):
                sps = mm.tile([128, HPC, 512], f32, tag="sps",
                              name=f"sps{qc}_{kb}_{b}")
                for h in range(HPC):
                    nc.tensor.matmul(
                        sps[:, h, :],
                        lhsT=kT_sb[:, b * S + kb * 128:b * S + (kb + 1) * 128],
                        rhs=qT_h[h][:, b * S + qc * 512:b * S + (qc + 1) * 512],
                        start=True, stop=True)
                g, half = kb // 2, kb % 2
                if half == 0:
                    est = esp.tile([128, 2, HPC, 512], f16, tag="es",
                                   name=f"es{qc}_{b}_{g}")
                nc.scalar.activation(est[:, half], sps, func=Exp, scale=SCALE)
                if half == 1:
                    ptt = ptp.tile([128, 2, HPC, 512], f16, tag="pt",
                                   name=f"pt{qc}_{b}_{g}")
                    ebt = eb_halves[g // 4]
                    gg = g % 4
                    nc.vector.tensor_mul(ptt, est,
                                         ebt[:, 2 * gg:2 * gg + 2, :, :])
                    pend.append((ptt, g))
                    if len(pend) > PIPE:
                        emit_pv(*pend.pop(0))
                if kb == 8 and next_half is not None:
                    next_half()
                pump(1)
            for args in pend:
                emit_pv(*args)

            # evacuate ctx rows to SBUF immediately: frees the cvp bank
            # pair well before the broadcast round-trip completes, so the
            # next chunk's PV accumulation is not serialized behind it.
            cvs = bcp.tile([64, HPC, 512], f32, tag="cvs",
                           name=f"cvs{qc}_{b}")
            nc.vector.tensor_copy(cvs, cvp[0:64, :, :])
            # 1/denominator via exp(-ln d) on the scalar engine (a DVE
            # reciprocal costs 6.5us per chunk; two tiny activations don't)
            rc_sb = rcp.tile([65, 2, HPC, 512], f32, tag="rc")
            Ln = Exp.__class__.Ln
            nc.scalar.activation(rc_sb[64:65, 0], cvp[64:65, :, :], func=Ln)
            nc.scalar.activation(rc_sb[64:65, 1], rc_sb[64:65, 0],
                                 func=Exp, scale=-1.0)
            scr = dscr.tile([1, HPC, 512], f32, tag="scr",
                            name=f"scr{qc}_{b}")
            nc.sync.dma_start(scr, rc_sb[64:65, 1])
            bc = bcp.tile([64, HPC, 512], f32, tag="bc")
            nc.sync.dma_start(bc, scr.to_broadcast((64, HPC, 512)))
            nc.vector.tensor_mul(
                ctxn[b][0:64, qc * 512:(qc + 1) * 512], cvs[:, 0, :],
                bc[:, 0, :])
            # h1: lanes 0-63; via ctx1, relocated to partitions 64-127
            nc.vector.tensor_mul(
                ctx1[b][:, qc * 512:(qc + 1) * 512], cvs[:, 1, :],
                bc[:, 1, :])
            nc.sync.dma_start(
                ctxn[b][64:128, qc * 512:(qc + 1) * 512],
                ctx1[b][:, qc * 512:(qc + 1) * 512])

        # ---------- schedule ----------
        nc.sync.dma_start(wq_sb, wq.rearrange("(f p) j -> p f j", p=128))
        nc.sync.dma_start(qb_sb, qb)
        nc.sync.dma_start(wk_sb, wk.rearrange("(f p) j -> p f j", p=128))
        nc.sync.dma_start(kb_sb, kb_)
        nc.sync.dma_start(wv_sb, wv.rearrange("(f p) j -> p f j", p=128))
        nc.sync.dma_start(wo_sb, wo)

        # bootstrap: just enough projection for attn(0,0) to start
        boot = [proj_unit("q", 0), proj_unit("k", 0), proj_unit("v", 0),
                proj_unit("k", 1), proj_unit("v", 1)]
        for u in boot[:2]:
            u["dma"]()
        for i, u in enumerate(boot):
            if i + 2 < len(boot):
                boot[i + 2]["dma"]()
            u["work"]()

        # filler queues per chunk
        units.extend([proj_unit("k", 2), proj_unit("v", 2),
                      proj_unit("k", 3), proj_unit("v", 3),
                      proj_unit("k", 4), proj_unit("v", 4),
                      proj_unit("k", 5), proj_unit("v", 5),
                      proj_unit("q", 4)])
        c1_fill = [proj_unit("k", 6), proj_unit("v", 6),
                   proj_unit("k", 7), proj_unit("v", 7),
                   proj_unit("q", 1), proj_unit("q", 2), proj_unit("q", 3)]
        c2_fill = [proj_unit("q", 5), proj_unit("q", 6), proj_unit("q", 7)]

        chunks = [(qc, b) for qc in range(QC) for b in range(B)]
        halves = {}
        halves[(0, 0)] = ebq_half(0, 0)
        halves[(0, 1)] = ebq_half(0, 1)

        for ci, (qc, b) in enumerate(chunks):
            if ci == 1:
                units.extend(c1_fill)
            elif ci == 2:
                units.extend(c2_fill)
            if ci >= 2:
                units.extend(op_units(*chunks[ci - 2]))

            def next_half(qc=qc, b=b):
                # during (qc, 1) prefetch the next qc's first half
                if b == 1 and qc + 1 < QC:
                    halves[(qc + 1, 0)] = ebq_half(qc + 1, 0)
            attn(qc, b, [halves[(qc, 0)], halves[(qc, 1)]], next_half)
            if b == 1 and qc + 1 < QC:
                halves[(qc + 1, 1)] = ebq_half(qc + 1, 1)

        units.extend(op_units(*chunks[-2]))
        units.extend(op_units(*chunks[-1]))
        drain()


def get_module():
    if "nc" not in _CACHE:
        _CACHE["nc"] = _build_module()
    return _CACHE["nc"]


def make_in_maps(query, key, value, key_padding_mask, bias,
                 q_w, q_b, k_w, k_b, v_w, v_b, o_w, o_b):
    f16 = np.float16
    xq_t = np.ascontiguousarray(query.reshape(R, H).T).astype(f16)
    xk_t = np.ascontiguousarray(key.reshape(R, H).T).astype(f16)
    xv_t = np.ascontiguousarray(value.reshape(R, H).T).astype(f16)

    kpm = np.asarray(key_padding_mask)  # [B, S] bool
    # m01[p, t]: 0.0 where key-padding-masked, column t = b*KB + kb
    m01 = np.empty((128, T), np.float32)
    for b in range(B):
        for kb in range(KB):
            m01[:, b * KB + kb] = np.where(kpm[b, kb * 128:(kb + 1) * 128],
                                           0.0, 1.0)
    m01_f32 = np.ascontiguousarray(m01)
    m01_v = m01.astype(f16)

    in_maps = []
    for c in range(NCORES):
        hs = slice(c * CW, (c + 1) * CW)
        # eb layout [qc, k, i, qi]: exp(bias).T pre-sliced by q chunk
        ebt = np.empty((QC, S, HPC, 512), f16)
        for i in range(HPC):
            h = c * HPC + i
            e = np.exp(np.asarray(bias[0, h], np.float32).T)  # [k, q]
            ebt[:, :, i, :] = e.reshape(S, QC, 512).transpose(1, 0, 2)
        ebt = ebt.reshape(QC, S, HPC * 512)
        in_maps.append({
            "xq_t": xq_t, "xk_t": xk_t, "xv_t": xv_t,
            "wq_t": np.ascontiguousarray(np.asarray(q_w)[hs].T).astype(f16),
            "wk_t": np.ascontiguousarray(np.asarray(k_w)[hs].T).astype(f16),
            "wv_t": np.ascontiguousarray(np.asarray(v_w)[hs].T).astype(f16),
            "wo_t": np.ascontiguousarray(np.asarray(o_w)[:, hs].T).astype(f16),
            "qb_col": np.asarray(q_b, np.float32)[hs].reshape(CW, 1).copy(),
            "kb_col": np.asarray(k_b, np.float32)[hs].reshape(CW, 1).copy(),
            "eb_t": ebt,
            "m01_f32": m01_f32,
            "m01_v": m01_v,
        })
    return in_maps


def assemble_output(results, v_b, o_w, o_b):
    acc = np.zeros((R, H), np.float32)
    for res in results:
        acc += np.asarray(res["o_part"], np.float32)
    corr = np.asarray(v_b, np.float32) @ np.asarray(o_w, np.float32).T \
        + np.asarray(o_b, np.float32)
    acc += corr[None, :]
    return acc.reshape(B, S, H).astype(np.float32)


def kernel(**inputs):
    from concourse.bass_utils import run_bass_kernel_spmd

    nc = get_module()
    in_maps = make_in_maps(**inputs)
    res = run_bass_kernel_spmd(nc, in_maps, list(range(NCORES)))
    return assemble_output(res.results, inputs["v_b"], inputs["o_w"],
                           inputs["o_b"])
